# revision 1
# baseline (speedup 1.0000x reference)
"""Trainium2 Bass kernel for AttentionWithRelPos.

Reference computation (fp32):
    qkv = x @ w_qkv.T                      # [B, N, 3C]
    q, k, v = split/reshape                # [B, H, N, HD]
    attn = softmax(q @ k.T * scale + bias) # bias gathered from rel_pos
    out  = (attn @ v).merge_heads @ w_proj.T + b_proj

Sharding: data-parallel over batch across 8 NeuronCores (8 batches/core).
All matmuls in bf16 with fp32 PSUM accumulation.

Transpose-free attention pipeline.  Instead of computing S = q@k.T,
softmaxing along the free dim, and PE-transposing P for the PV matmul,
each (batch, head) unit computes S TRANSPOSED directly (k on partitions):
  1. qkT = WqkT.T-stationary @ xT           -> [1536, tok] (q rows scaled)
  2. v   = xT-stationary @ WvT              -> per-(b,kt) [<=128, 12*65]
     (65th column of each head slot is a constant-1 column -> the PV
      matmul emits the softmax row-sum for free as output row 64)
  3. per (b, head-pair):
     sT[k,q] = kT-slice.T-stationary @ qT-slice  (2 k-tiles into one PSUM
     bank [128,2,197]; the kt=1 stationary spans zero-padded qkT columns
     so every PSUM partition is written)
     t1 = exp(sT)            (one ACT op per head; no max subtraction --
                              logits are O(1) for this problem)
     eT = t1 * expbiasT      (one DVE op per pair, all-bf16; host
                              precomputes exp(bias).T so no bias matmul
                              and no separate add pass is needed)
     o[0:65,q] = v-slice(+ones).T-stationary @ eT   (row 64 = rowsum r;
     both heads share one PSUM bank side by side)
     rcp = 1/r               (one DVE reciprocal over both heads' rows)
     bc  = ones64.T @ rcp    (one rank-1 PE matmul broadcasts both rows
                              onto partitions 64:128 of the same bank)
     attT-slice = o * bc_sb  (bc staged to SBUF on ACT -- DVE may read
                              only one PSUM operand -- then 2 DVE mults)
  4. y = attT.T-stationary @ WpT            -> [tok, 768] -> DRAM
Attention is software-pipelined 3 stages deep (S^T+exp+eT, then PV+rcp
two pair-slots later, then bcast+normalize one slot after that) so PE
never parks on an instruction whose ACT/DVE producers are still running.
Emission is diagonal-wave interleaved (qk-proj chunk-pairs, per-batch
v-proj, attention, and trailing proj chunks all overlap); input DMAs are
ordered/split so the first accumulation chain starts ~3 us in.
Host adds b_proj and re-assembles [64, 197, 768].
"""

import sys

if "/opt/trn_rl_repo" not in sys.path:
    sys.path.insert(0, "/opt/trn_rl_repo")

import numpy as np
import ml_dtypes

BF16 = ml_dtypes.bfloat16

B, DIM, HEADS, N = 64, 768, 12, 197
HD = DIM // HEADS  # 64
SCALE = HD ** -0.5
NCORES = 8
BL = B // NCORES  # 8 batches per core
KC = DIM // 128  # 6 contraction chunks
HSLOT = HD + 1   # 65: v columns per head incl. the ones column

_CACHE = {}


def _build(bl=BL, probe=4):
    """Build + compile the per-core Bass program. Returns the compiled nc.

    probe: debug level — 0 skips attention; 4 full.
    """
    import concourse.bacc as bacc
    import concourse.bass as bass
    import concourse.tile as tile
    from concourse import mybir
    from contextlib import ExitStack

    f32 = mybir.dt.float32
    bf16 = mybir.dt.bfloat16
    ALU = mybir.AluOpType
    ACTF = mybir.ActivationFunctionType

    tok = bl * N

    nc = bacc.Bacc("TRN2", target_bir_lowering=False, debug=False,
                   enable_asserts=False, num_devices=NCORES)

    # all inputs are laid out partition-major, exactly matching their SBUF
    # destination, so one DMA can cover an arbitrary sub-range
    xT = nc.dram_tensor("xT", (128, KC, tok), bf16, kind="ExternalInput").ap()
    wqkT = nc.dram_tensor("wqkT", (128, KC, 2 * DIM), bf16,
                          kind="ExternalInput").ap()
    wvT = nc.dram_tensor("wvT", (128, KC, DIM), bf16, kind="ExternalInput").ap()
    wpT = nc.dram_tensor("wpT", (128, KC, DIM), bf16, kind="ExternalInput").ap()
    ebT = nc.dram_tensor("ebT", (128, HEADS, 2, N), bf16,
                         kind="ExternalInput").ap()
    y = nc.dram_tensor("y", (tok, DIM), f32, kind="ExternalOutput").ap()

    # token-chunking for matmul moving dims
    NCH = 4 if tok % 4 == 0 else 1   # qk-proj rhs chunks
    CH = tok // NCH                  # 394 for bl=8
    assert CH <= 512
    # proj m-tiles (dense 128-token chunks)
    mt_sizes = [128] * (tok // 128) + ([tok % 128] if tok % 128 else [])
    qt_sizes = [128, N - 128]

    with ExitStack() as ctx:
        tc = ctx.enter_context(tile.TileContext(nc))
        singles = ctx.enter_context(tc.tile_pool(name="singles", bufs=1))
        mm_psum = ctx.enter_context(tc.tile_pool(name="mm_psum", bufs=3, space="PSUM"))
        s_psum = ctx.enter_context(tc.tile_pool(name="s_psum", bufs=2, space="PSUM"))
        o_psum = ctx.enter_context(tc.tile_pool(name="o_psum", bufs=3, space="PSUM"))
        work = ctx.enter_context(tc.tile_pool(name="work", bufs=5))
        stats = ctx.enter_context(tc.tile_pool(name="stats", bufs=10))

        # ---- persistent SBUF tensors ----
        NPAD = 256  # per-batch column slot in qkT (cols N..NPAD are zero)
        xT_sb = singles.tile([128, KC, tok], bf16)
        wqk_sb = singles.tile([128, KC, 2 * DIM], bf16)
        wv_sb = singles.tile([128, KC, DIM], bf16)
        wp_sb = singles.tile([128, KC, DIM], bf16)
        ebT_sb = singles.tile([128, HEADS, 2, N], bf16)
        qkT_sb = singles.tile([128, 2 * KC, bl, NPAD], bf16)
        v_sb = singles.tile([128, bl, 2, HEADS, HSLOT], bf16)
        attT_sb = singles.tile([128, KC, tok], bf16)
        ones_sb = singles.tile([1, HD], bf16)
        NHP = HEADS // 2

        # ---- input DMAs, ordered so the first qk-proj wave (m-tiles 0 and
        # KC) and the first batch's v-proj/attention unblock ASAP.  DRAM
        # layouts mirror SBUF, so each transfer is a single DMA. ----
        def dma_x_chunk(n):
            nc.sync.dma_start(out=xT_sb[:, :, n * CH:(n + 1) * CH],
                              in_=xT[:, :, n * CH:(n + 1) * CH])

        def dma_wqk_mpair(j):
            for m in (j, KC + j):
                nc.sync.dma_start(
                    out=wqk_sb[:, :, m * 128:(m + 1) * 128],
                    in_=wqkT[:, :, m * 128:(m + 1) * 128])

        def dma_eb(h0, h1):
            nc.sync.dma_start(out=ebT_sb[:, h0:h1, :, :], in_=ebT[:, h0:h1, :, :])

        dma_x_chunk(0)
        dma_wqk_mpair(0)
        if NCH > 1:
            dma_x_chunk(1)
        dma_wqk_mpair(1)
        nc.sync.dma_start(out=wv_sb[:, :, :], in_=wvT[:, :, :])
        dma_eb(0, 2)
        for n in range(2, NCH):
            dma_x_chunk(n)
            dma_wqk_mpair(n)
        for j in range(max(NCH, 2), NHP):
            if j == NHP - 2:
                dma_eb(2, 6)
            dma_wqk_mpair(j)
        dma_eb(6, HEADS)
        nc.sync.dma_start(out=wp_sb[:, :, :], in_=wpT[:, :, :])

        # constants: PV ones column + broadcast stationary + qkT padding
        nc.vector.memset(ones_sb[:, :], 1.0)
        nc.gpsimd.memset(qkT_sb[:, :, :, N:NPAD], 0.0)
        nc.gpsimd.memset(v_sb[:, :, :, :, HD:HSLOT], 1.0)

        assert CH % N == 0
        bpc = CH // N  # batches per qk-proj chunk

        def emit_qkproj_chunk(m, n):
            ps = mm_psum.tile([128, bpc, N], f32, tag="mm", name="ps")
            for kc in range(KC):
                nc.tensor.matmul(
                    ps[:, :, :],
                    lhsT=wqk_sb[:, kc, m * 128:(m + 1) * 128],
                    rhs=xT_sb[:, kc, n * CH:(n + 1) * CH],
                    start=(kc == 0), stop=(kc == KC - 1),
                )
            dst = qkT_sb[:, m, n * bpc:(n + 1) * bpc, 0:N]
            nc.vector.tensor_copy(dst, ps[:, :, :])

        def emit_qkproj_wave(j):
            # chunk-major across the q/k m-tile pair: follows the x-chunk
            # DMA arrival order, so wave 0 can start ~2 MB sooner
            for n in range(NCH):
                emit_qkproj_chunk(j, n)
                emit_qkproj_chunk(KC + j, n)

        def emit_vproj(b, kts=(0, 1)):
            for kt in kts:
                rows = qt_sizes[kt]
                t0 = b * N + kt * 128
                for n2 in range(2):
                    ps = mm_psum.tile([128, 8, HD], f32, tag="mm", name="ps")
                    for kc in range(KC):
                        nc.tensor.matmul(
                            ps[0:rows, 0:6, :],
                            lhsT=xT_sb[:, kc, t0:t0 + rows],
                            rhs=wv_sb[:, kc, n2 * 384:(n2 + 1) * 384],
                            start=(kc == 0), stop=(kc == KC - 1),
                        )
                    # scatter the 6 head-blocks past the ones columns.
                    # early batches evacuate on DVE (nearly idle in the
                    # first waves, while ACT is busy with the first exps)
                    dst = v_sb[0:rows, b, kt, n2 * 6:(n2 + 1) * 6, 0:HD]
                    if b < 2:
                        nc.vector.tensor_copy(dst, ps[0:rows, 0:6, :])
                    else:
                        nc.scalar.copy(out=dst, in_=ps[0:rows, 0:6, :])

        # Attention is software-pipelined in three stages so PE never has to
        # sit at an instruction whose producers (ACT exp / DVE) are still
        # running: each pair's PV lands three pair-slots after its S^T, and
        # its rcp-broadcast one slot after that.
        def attn_stage1(b, j):
            """S^T / exp for both heads of pair j, one fused *expbias.

            Returns the pair's eT tile [128, head, kt, N]."""
            t1 = work.tile([128, 2, 2, N], bf16, tag="t1", name="t1", bufs=6)
            for i in range(2):
                po = i * 64
                qT = qkT_sb[po:po + 64, j, b, 0:N]
                kT = qkT_sb[po:po + 64, KC + j, b, :]
                # sT[k, q] both k-tiles into one PSUM bank; the kt=1
                # stationary includes the zero padding columns so all 128
                # output partitions are written (keeps PSUM reads fully
                # initialized for the exp)
                s = s_psum.tile([128, 2, N], f32, tag="s", name="s")
                for kt in range(2):
                    nc.tensor.matmul(
                        s[:, kt, :],
                        lhsT=kT[:, kt * 128:(kt + 1) * 128],
                        rhs=qT,
                        start=True, stop=True,
                    )
                if probe < 2:
                    continue
                # exp (no max subtraction: |logits| = O(1) by construction)
                nc.scalar.activation(out=t1[:, i, :, :], in_=s[:, :, :],
                                     func=ACTF.Exp)
            if probe < 2:
                return None
            # * exp(bias).T (host-precomputed), both heads in one DVE op
            eT = work.tile([128, 2, 2, N], bf16, tag="eT", name="eT", bufs=6)
            nc.vector.tensor_tensor(eT[:, :, :, :], t1[:, :, :, :],
                                    ebT_sb[:, 2 * j:2 * j + 2, :, :], ALU.mult)
            return eT

        def attn_stage2(b, j, eT):
            """PV for both heads into one shared PSUM bank + one reciprocal.

            Bank layout (f32): head A output on partitions 0:65 cols 0:197,
            head B on partitions 0:65 cols 197:394.  The rcp broadcast later
            lands on partitions 64:128 of the same columns — only partition 64
            (the dead row-sum row, already consumed by the reciprocal)
            overlaps.
            """
            if probe < 3:
                return None
            o = o_psum.tile([128, 512], f32, tag="o", name="o")
            for i in range(2):
                for kt in range(2):
                    kn = qt_sizes[kt]
                    nc.tensor.matmul(
                        o[0:HSLOT, i * N:i * N + N],
                        lhsT=v_sb[0:kn, b, kt, 2 * j + i, :],
                        rhs=eT[0:kn, i, kt, :],
                        start=(kt == 0), stop=(kt == 1),
                    )
            rcp2 = stats.tile([1, 2 * N], bf16, tag="rcp")
            with nc.allow_low_precision("bf16 softmax normalizer"):
                nc.vector.reciprocal(rcp2[:, :], o[HD:HSLOT, 0:2 * N])
            return o, rcp2

        done3 = [0] * bl  # stage3 completions per batch (proj waterline)

        def attn_stage3(b, j, o, rcp2):
            """rcp broadcast (PE) + staging copy + normalize/evacuate."""
            done3[b] += 1
            # one rank-1 matmul broadcasts BOTH heads' rcp rows at once: the
            # bc region o[64:128, 0:394] is rcp2's 394-wide row replicated
            # onto every partition
            nc.tensor.matmul(
                o[64:128, 0:2 * N],
                lhsT=ones_sb[:, :],
                rhs=rcp2[:, :],
                start=True, stop=True,
                skip_group_check=True,
            )
            if probe < 4:
                return
            # DVE may read only ONE operand from PSUM: stage bc in SBUF (ACT)
            bc_sb = stats.tile([64, 2 * N], bf16, tag="bc")
            nc.scalar.copy(out=bc_sb[:, :], in_=o[64:128, 0:2 * N])
            # normalize + evacuate (GPSIMD has no PSUM port -> DVE)
            for i in range(2):
                dst = attT_sb[i * 64:i * 64 + 64, j, b * N:(b + 1) * N]
                nc.vector.tensor_tensor(dst, o[0:HD, i * N:(i + 1) * N],
                                        bc_sb[:, i * N:(i + 1) * N],
                                        ALU.mult)

        pend = []  # entries: [age, b, j, payload]; stage2 at age 3, 3 at 4

        def attn_advance(pair=None):
            for entry in list(pend):
                entry[0] += 1
                if entry[0] == 3:
                    entry[3] = attn_stage2(entry[1], entry[2], entry[3])
                elif entry[0] == 4:
                    if entry[3] is not None:
                        attn_stage3(entry[1], entry[2], *entry[3])
                    pend.remove(entry)
            if pair is not None:
                b, j = pair
                eT = attn_stage1(b, j)
                if probe >= 2:
                    pend.append([0, b, j, eT])


        def emit_proj(mt):
            rows = mt_sizes[mt]
            t0 = mt * 128
            for n2 in range(2):
                ps = mm_psum.tile([128, 512], f32, tag="mm", name="ps")
                for kc in range(KC):
                    nc.tensor.matmul(
                        ps[0:rows, 0:384],
                        lhsT=attT_sb[:, kc, t0:t0 + rows],
                        rhs=wp_sb[:, kc, n2 * 384:(n2 + 1) * 384],
                        start=(kc == 0), stop=(kc == KC - 1),
                    )
                yst = work.tile([128, 384], f32, tag="yst")
                nc.scalar.copy(out=yst[0:rows, :], in_=ps[0:rows, 0:384])
                nc.sync.dma_start(
                    out=y[t0:t0 + rows, n2 * 384:(n2 + 1) * 384],
                    in_=yst[0:rows, :],
                )

        # ---- emission: diagonal wave (see module docstring) ----
        proj_ptr = [0]

        def emit_proj_upto(limit):
            while proj_ptr[0] < limit:
                emit_proj(proj_ptr[0])
                proj_ptr[0] += 1

        if probe >= 1:
            for w in range(bl + NHP - 1):
                # interleave this wave's qk-proj chunk-pairs with its
                # attention slots so DVE/ACT load is spread evenly
                chunks = ([(m, n) for n in range(NCH) for m in (w, KC + w)]
                          if w < NHP else [])
                slots = [(b, w - b) for b in range(bl) if 0 <= w - b < NHP]
                ci = 0
                for si, (b, hp) in enumerate(slots):
                    want = ((si + 1) * len(chunks) + len(slots) - 1) // len(slots) if slots else 0
                    while ci < min(want, len(chunks)):
                        emit_qkproj_chunk(*chunks[ci])
                        ci += 1
                    if hp == 0:
                        emit_vproj(b, kts=(0,))
                    attn_advance((b, hp))
                    if hp == 0:
                        emit_vproj(b, kts=(1,))
                while ci < len(chunks):
                    emit_qkproj_chunk(*chunks[ci])
                    ci += 1
                nb = 0
                while nb < bl and done3[nb] == NHP:
                    nb += 1
                emit_proj_upto((nb * N) // 128)
            while pend:
                attn_advance(None)
                nb = 0
                while nb < bl and done3[nb] == NHP:
                    nb += 1
                emit_proj_upto((nb * N) // 128)
            emit_proj_upto(len(mt_sizes))
        else:
            for m in range(2 * KC):
                for n in range(NCH):
                    emit_qkproj_chunk(m, n)
            for b in range(bl):
                emit_vproj(b)
            nc.vector.memset(attT_sb[:, :, :], 0.0)
            for mt in range(len(mt_sizes)):
                emit_proj(mt)

    nc.compile()
    return nc


def _prep_shared(w_qkv, w_proj, rel_pos, rel_pos_index):
    """Host-side input prep shared across cores (weights / exp-bias)."""
    w_qkv = np.asarray(w_qkv, dtype=np.float32)
    w_proj = np.asarray(w_proj, dtype=np.float32)
    rel_pos = np.asarray(rel_pos, dtype=np.float32)
    rel_pos_index = np.asarray(rel_pos_index)

    def pmajor(wT):
        # [DIM, cols] -> partition-major [128, KC, cols]
        return np.ascontiguousarray(
            wT.reshape(KC, 128, wT.shape[1]).swapaxes(0, 1)).astype(BF16)

    wqk = w_qkv[:2 * DIM].copy()
    wqk[:DIM] *= SCALE  # fold attention scale into Wq
    wqkT = pmajor(np.ascontiguousarray(wqk.T))
    wvT = pmajor(np.ascontiguousarray(w_qkv[2 * DIM:].T))
    wpT = pmajor(np.ascontiguousarray(w_proj.T))

    bias_full = np.zeros((HEADS, N, N), dtype=np.float32)
    bias_full[:, 1:, 1:] = rel_pos[:, rel_pos_index]
    ebiasT = np.exp(bias_full.transpose(0, 2, 1))      # [H, k, q]
    ebT = np.zeros((128, HEADS, 2, N), dtype=np.float32)
    ebT[:, :, 0, :] = ebiasT[:, 0:128, :].transpose(1, 0, 2)
    ebT[0:N - 128, :, 1, :] = ebiasT[:, 128:N, :].transpose(1, 0, 2)
    return {"wqkT": wqkT, "wvT": wvT, "wpT": wpT, "ebT": ebT.astype(BF16)}


def _prep_core(x, core, bl=BL):
    """Per-core xT: partition-major [128, KC, bl*N] bf16."""
    xc = np.asarray(x[core * bl:(core + 1) * bl], dtype=np.float32)
    xT = xc.reshape(bl * N, DIM).T.reshape(KC, 128, bl * N)
    return np.ascontiguousarray(xT.swapaxes(0, 1)).astype(BF16)


def kernel(x, w_qkv, w_proj, b_proj, rel_pos, rel_pos_index):
    from concourse.bass_utils import run_bass_kernel_spmd

    x = np.asarray(x, dtype=np.float32)
    w_qkv = np.asarray(w_qkv, dtype=np.float32)
    w_proj = np.asarray(w_proj, dtype=np.float32)
    b_proj = np.asarray(b_proj, dtype=np.float32)
    rel_pos = np.asarray(rel_pos, dtype=np.float32)
    rel_pos_index = np.asarray(rel_pos_index)

    if "nc" not in _CACHE:
        _CACHE["nc"] = _build(BL)
    nc = _CACHE["nc"]

    shared = _prep_shared(w_qkv, w_proj, rel_pos, rel_pos_index)
    in_maps = []
    for core in range(NCORES):
        m = dict(shared)
        m["xT"] = _prep_core(x, core)
        in_maps.append(m)

    try:
        y_cores = _run_cached(nc, in_maps)
    except Exception:
        res = run_bass_kernel_spmd(nc, in_maps, core_ids=list(range(NCORES)))
        y_cores = [r["y"] for r in res.results]
    y = np.concatenate(
        [yc.reshape(BL, N, DIM) for yc in y_cores], axis=0
    ).astype(np.float32)
    return y + b_proj[None, None, :]


def _run_cached(nc, in_maps):
    """Execute via a cached jitted shard_map executable (run_bass_kernel_spmd
    re-traces per call; this path pays tracing/lowering only once)."""
    import jax
    from jax.sharding import Mesh, PartitionSpec, NamedSharding
    from jax.experimental.shard_map import shard_map
    from concourse import bass2jax, mybir

    if "exe" not in _CACHE:
        bass2jax.install_neuronx_cc_hook()
        pname = nc.partition_id_tensor.name if nc.partition_id_tensor else None
        in_names, out_names, out_avals, zeros = [], [], [], []
        for alloc in nc.m.functions[0].allocations:
            if not isinstance(alloc, mybir.MemoryLocationSet):
                continue
            name = alloc.memorylocations[0].name
            if alloc.kind == "ExternalInput":
                if name != pname:
                    in_names.append(name)
            elif alloc.kind == "ExternalOutput":
                out_names.append(name)
                shape = tuple(alloc.tensor_shape)
                dtype = mybir.dt.np(alloc.dtype)
                out_avals.append(jax.core.ShapedArray(shape, dtype))
                zeros.append(np.zeros(shape, dtype))
        n_params = len(in_names)
        all_in = in_names + out_names + ([pname] if pname else [])

        def _body(*args):
            operands = list(args)
            if pname is not None:
                operands.append(bass2jax.partition_id_tensor())
            return tuple(bass2jax._bass_exec_p.bind(
                *operands, out_avals=tuple(out_avals), in_names=tuple(all_in),
                out_names=tuple(out_names), lowering_input_output_aliases=(),
                sim_require_finite=True, sim_require_nnan=True, nc=nc))

        devices = jax.devices()[:NCORES]
        mesh = Mesh(np.asarray(devices), ("core",))
        n_outs = len(out_names)
        sharded = jax.jit(
            shard_map(_body, mesh=mesh,
                      in_specs=(PartitionSpec("core"),) * (n_params + n_outs),
                      out_specs=(PartitionSpec("core"),) * n_outs,
                      check_rep=False),
            keep_unused=True,
        )
        sh = NamedSharding(mesh, PartitionSpec("core"))
        zero_dev = [
            jax.device_put(
                np.zeros((NCORES * z.shape[0], *z.shape[1:]), z.dtype), sh)
            for z in zeros
        ]
        _CACHE["exe"] = (sharded, in_names, out_names, zero_dev, sh)

    sharded, in_names, out_names, zero_dev, sh = _CACHE["exe"]
    concat_in = [
        np.concatenate([np.asarray(in_maps[c][nm]) for c in range(NCORES)],
                       axis=0)
        for nm in in_names
    ]
    out = sharded(*[jax.device_put(a, sh) for a in concat_in], *zero_dev)
    yi = out_names.index("y")
    y_all = np.asarray(out[yi])
    rows = y_all.shape[0] // NCORES
    return [y_all[c * rows:(c + 1) * rows] for c in range(NCORES)]

